# revision 1
# baseline (speedup 1.0000x reference)
"""Trainium2 Bass kernel for nn_Encoder_79096117723504 (gnn_message_passing).

Key algebraic insight: the reference gathers its 2048 edges out of a 512-row
node table, so every edge-level quantity is a gather of a node-level one.
The [H,F,T] edge attention collapses exactly to node space:

  softmax over the 2048 'to' edges == multiplicity-weighted softmax over the
  512 unique 'to' nodes (weights ct[v] = histogram of tpos), and the final
  mean over the 2048 'from' edges == (cf @ LN_out) / 2048 with cf = histogram
  of fpos.

This cuts ~618 GFLOP of edge-level work to ~56 GFLOP of node-level work.
Work splits across 8 NeuronCores as 6 independent (segment, direction)
sub-problems (cores 6,7 run redundant duplicates); each core returns one
[768] mean-pooled vector; the host concatenates them into [3, 1536].

Matmul operands are fp16 (full PE rate; node ids <= 511 and histogram counts
are exact in fp16; everything accumulates in fp32 PSUM).  Per-core device
program:
  hists        : ct,cf via PE outer-product broadcast + iota-compare + reduce
  projections  : qT,kT d-major; v,cb node-major; biases folded via ones-row
  per head h   : kmT = kT * mix_h; S^T[v,u] (768-contraction);
                 E = exp(S^T/SCALE + cb_v/SCALE)  (ACT, per-partition bias);
                 ctx^T[64,u] = vct_h^T @ E,  Z = ct^T @ E;
                 ctx rows scaled by 1/Z (vector recip + PE row-broadcast)
  epilogue     : x = h + ctx @ Wd^T + bd (u-major); LayerNorm along free dim;
                 o = (cf @ y) / 2048 via count-weight matmul.
All partition offsets are 32-aligned (heads live in 64-row padded blocks).
"""
import math
import os
import sys
from contextlib import ExitStack

import numpy as np

for p in ('/opt/trn_rl_repo', '/root/.axon_site/_ro/trn_rl_repo'):
    if os.path.isdir(p) and p not in sys.path:
        sys.path.insert(0, p)

import concourse.bass as bass
import concourse.mybir as mybir
from concourse import bacc, tile
from concourse.bass_utils import run_bass_kernel_spmd

F32 = mybir.dt.float32
F16 = mybir.dt.float16
I32 = mybir.dt.int32

D = 768
H = 16
DH = 48
SEQ = 512
NE = 2048
EPS = 1e-5
SCALE = math.sqrt(D / H)
KT = 6            # 768 / 128 k-tiles
VT = 4            # 512 / 128 v(or u)-tiles
PADK = H * 64     # padded ctx contraction dim (16 heads x 64)

_NC_CACHE = {}


def _mm(nc, out, lhsT, rhs, start, stop):
    nc.tensor.matmul(out, lhsT, rhs, start=start, stop=stop)


def build_nc():
    """One SPMD program; per-core data selects the (segment, direction)."""
    # Bacc (not raw Bass): its compile pipeline legalizes multi-sem waits
    # (split_sync_waits) and auto-inserts gpsimd library loads for walrus.
    nc = bacc.Bacc(None, target_bir_lowering=False)

    # -------- DRAM I/O (parameters are bound by position) -------------------
    hTe = nc.declare_dram_parameter("hTe", [D + 1, SEQ], F16, isOutput=False)
    h_nm = nc.declare_dram_parameter("h_nm", [SEQ, D], F32, isOutput=False)
    WqT = nc.declare_dram_parameter("WqT", [D, D], F16, isOutput=False)
    WkT = nc.declare_dram_parameter("WkT", [D, D], F16, isOutput=False)
    WvTe = nc.declare_dram_parameter("WvTe", [D + 1, D], F16, isOutput=False)
    WdTe = nc.declare_dram_parameter("WdTe", [PADK + 1, D], F16, isOutput=False)
    WcbT = nc.declare_dram_parameter("WcbT", [D, H], F16, isOutput=False)
    mixT = nc.declare_dram_parameter("mixT", [D, H], F32, isOutput=False)
    lng = nc.declare_dram_parameter("lng", [1, D], F16, isOutput=False)
    lnb = nc.declare_dram_parameter("lnb", [1, D], F16, isOutput=False)
    frpos = nc.declare_dram_parameter("frpos", [1, NE], I32, isOutput=False)
    topos = nc.declare_dram_parameter("topos", [1, NE], I32, isOutput=False)
    o_out = nc.declare_dram_parameter("o", [1, D], F32, isOutput=True)
    dbg = nc.declare_dram_parameter("dbg", [128, 2 * VT], F32, isOutput=True)

    with tile.TileContext(nc) as tc:
        with (
            tc.tile_pool(name="const", bufs=1) as cpool,          # long-lived SBUF
            tc.tile_pool(name="psA", bufs=5, space="PSUM") as psA,  # [128,512] banks
            tc.tile_pool(name="psB", bufs=3, space="PSUM") as psB,  # small outs
        ):
            # ---------------- bulk loads: one 3D-AP DMA per tensor ----------
            hT_sb = cpool.tile([128, KT, SEQ], F16, tag="hT")
            nc.sync.dma_start(hT_sb[:],
                              hTe[0:D, :].rearrange("(t p) u -> p t u", p=128))
            ones_sb = cpool.tile([1, SEQ], F16, tag="ones")
            nc.sync.dma_start(ones_sb[:], hTe[D:D + 1, :])
            h_nm_sb = cpool.tile([128, VT, D], F32, tag="h_nm")
            nc.sync.dma_start(h_nm_sb[:],
                              h_nm[:, :].rearrange("(t p) d -> p t d", p=128))
            wq_sb = cpool.tile([128, KT, D], F16, tag="wq")
            nc.sync.dma_start(wq_sb[:],
                              WqT[:, :].rearrange("(t p) d -> p t d", p=128))
            wk_sb = cpool.tile([128, KT, D], F16, tag="wk")
            nc.sync.dma_start(wk_sb[:],
                              WkT[:, :].rearrange("(t p) d -> p t d", p=128))
            wv_sb = cpool.tile([128, KT, D], F16, tag="wv")
            nc.sync.dma_start(wv_sb[:],
                              WvTe[0:D, :].rearrange("(t p) d -> p t d", p=128))
            wd_sb = cpool.tile([128, 8, D], F16, tag="wd")
            nc.sync.dma_start(wd_sb[:],
                              WdTe[0:PADK, :].rearrange("(t p) d -> p t d", p=128))
            wcb_sb = cpool.tile([128, KT, H], F16, tag="wcb")
            nc.sync.dma_start(wcb_sb[:],
                              WcbT[:, :].rearrange("(t p) h -> p t h", p=128))
            mix_sb = cpool.tile([128, KT, H], F32, tag="mix")
            nc.sync.dma_start(mix_sb[:],
                              mixT[:, :].rearrange("(t p) h -> p t h", p=128))
            bv_sb = cpool.tile([1, D], F16, tag="bv")
            nc.sync.dma_start(bv_sb[:], WvTe[D:D + 1, :])
            bd_sb = cpool.tile([1, D], F16, tag="bd")
            nc.sync.dma_start(bd_sb[:], WdTe[PADK:PADK + 1, :])
            lng_b = cpool.tile([128, D], F16, tag="lng_b")
            lnb_b = cpool.tile([128, D], F16, tag="lnb_b")

            # ---------------- histograms ct (topos) / cf (frpos) ------------
            # counts col t = counts for node ids [128t, 128t+128).
            # [1,N] -> [P,N] broadcasts are K=1 outer products on the PE.
            ct_cols = cpool.tile([128, VT], F32, tag="ct")
            cf_cols = cpool.tile([128, VT], F32, tag="cf")
            cf_h16 = cpool.tile([128, VT], F16, tag="cf_h16")
            # iota_f[p, t] = p + 128*t — node id of partition p in v-tile t
            iota_f = cpool.tile([128, VT], F32, tag="iota_f")
            with tc.tile_pool(name="hist", bufs=1) as hpool:
                lng_row = hpool.tile([1, D], F16, tag="lng_row")
                lnb_row = hpool.tile([1, D], F16, tag="lnb_row")
                nc.sync.dma_start(lng_row[:], lng[:])
                nc.sync.dma_start(lnb_row[:], lnb[:])
                nc.gpsimd.partition_broadcast(lng_b[:], lng_row[:])
                nc.gpsimd.partition_broadcast(lnb_b[:], lnb_row[:])
                iota_i = hpool.tile([128, VT], I32, tag="iota_i")
                nc.gpsimd.iota(iota_i[:], pattern=[[128, VT]], base=0,
                               channel_multiplier=1)
                nc.vector.tensor_copy(iota_f[:], iota_i[:])
                for (pos_dram, cols) in ((topos, ct_cols), (frpos, cf_cols)):
                    pos_i = hpool.tile([1, NE], I32, tag="pos_i", bufs=2)
                    pos_h = hpool.tile([1, NE], F16, tag="pos_h", bufs=2)
                    eq_scr = hpool.tile([128, 512], F32, tag="eq_scr", bufs=2)
                    parts = hpool.tile([128, VT], F32, tag="parts", bufs=2)
                    nc.sync.dma_start(pos_i[:], pos_dram[:])
                    # pos % 512 == pos & 511 (ids are non-negative)
                    nc.vector.tensor_scalar(pos_i[:], pos_i[:], 511, None,
                                            op0=mybir.AluOpType.bitwise_and)
                    nc.vector.tensor_copy(pos_h[:], pos_i[:])   # exact: ids<=511
                    posb_h = hpool.tile([128, NE], F16, tag="posb_h", bufs=2)
                    nc.gpsimd.partition_broadcast(posb_h[:], pos_h[:])
                    for t in range(VT):
                        for c in range(4):
                            nc.vector.tensor_scalar(
                                eq_scr[:], posb_h[:, c * 512:(c + 1) * 512],
                                iota_f[:, t:t + 1], None,
                                op0=mybir.AluOpType.is_equal,
                                op1=mybir.AluOpType.add,
                                accum_out=parts[:, c:c + 1])
                        nc.vector.reduce_sum(cols[:, t:t + 1], parts[:],
                                             axis=mybir.AxisListType.X)
                nc.vector.tensor_copy(cf_h16[:], cf_cols[:])

            # ---------------- projections ----------------
            # qT,kT: [d,u] d-major; W streamed by k-tile from DRAM.
            qT_sb = cpool.tile([128, KT, SEQ], F16, tag="qT")
            kT_sb = cpool.tile([128, KT, SEQ], F16, tag="kT")
            for (wsb, dest) in ((wq_sb, qT_sb), (wk_sb, kT_sb)):
                for m in range(KT):
                    ps = psA.tile([128, SEQ], F32, tag="psA")
                    for k in range(KT):
                        _mm(nc, ps[:], wsb[:, k, m * 128:(m + 1) * 128],
                            hT_sb[:, k, :], start=(k == 0), stop=(k == KT - 1))
                    nc.vector.tensor_copy(dest[:, m, :], ps[:])

            # v node-major with bias, scaled by ct; head h lives in a 64-wide
            # block laid out [v dims 0:32 | ct | v dims 32:48 | 15 zeros] so
            # the softmax normalizer Z lands on the 32-aligned PSUM row 32 of
            # the ctx matmul (the host permutes Wd rows to match, with zero
            # rows under ct/padding).
            vct_sb = cpool.tile([128, VT, PADK], F16, tag="vct")
            nc.vector.memset(vct_sb[:], 0.0)
            for t in range(VT):
                ctcol16 = vct_sb[:, t, :].rearrange("p (h c) -> p h c", c=64)[:, :, 32]
                nc.vector.tensor_copy(
                    ctcol16, ct_cols[:, t:t + 1].to_broadcast((128, H)))
                for ns, (c0, c1) in enumerate(((0, 384), (384, 768))):
                    ps = psA.tile([128, 384], F32, tag="psA")
                    for k in range(KT):
                        _mm(nc, ps[:], hT_sb[:, k, t * 128:(t + 1) * 128],
                            wv_sb[:, k, c0:c1], start=(k == 0), stop=False)
                    _mm(nc, ps[:], ones_sb[:, t * 128:(t + 1) * 128],
                        bv_sb[:, c0:c1], start=False, stop=True)
                    for hh in range(8 * ns, 8 * ns + 8):
                        nc.vector.tensor_scalar(
                            vct_sb[:, t, hh * 64:hh * 64 + 32],
                            ps[:, hh * DH - c0:hh * DH - c0 + 32],
                            ct_cols[:, t:t + 1], None,
                            op0=mybir.AluOpType.mult)
                        nc.vector.tensor_scalar(
                            vct_sb[:, t, hh * 64 + 33:hh * 64 + 49],
                            ps[:, hh * DH - c0 + 32:(hh + 1) * DH - c0],
                            ct_cols[:, t:t + 1], None,
                            op0=mybir.AluOpType.mult)

            # content bias, node-major, pre-scaled by 1/SCALE (ACT bias = f32)
            cbs_sb = cpool.tile([128, VT, H], F32, tag="cbs")
            for t in range(VT):
                ps = psB.tile([128, H], F32, tag="psB")
                for k in range(KT):
                    _mm(nc, ps[:], hT_sb[:, k, t * 128:(t + 1) * 128],
                        wcb_sb[:, k, :], start=(k == 0), stop=(k == KT - 1))
                nc.vector.tensor_scalar(cbs_sb[:, t, :], ps[:], 1.0 / SCALE, None,
                                        op0=mybir.AluOpType.mult)

            # ---------------- attention heads ----------------
            # ctxT: padded d-major ctx, head h -> k-tile h//2, partition
            # offset 64*(h%2), 48 real + 16 zero rows per head.
            ctxT_sb = cpool.tile([128, 8, SEQ], F16, tag="ctxT")
            hstack = ExitStack()
            headpool = hstack.enter_context(tc.tile_pool(name="head", bufs=2))
            for hh in range(H):
                kmT = headpool.tile([128, KT, SEQ], F16, tag="kmT", bufs=1)
                for k in range(KT):
                    eng = nc.vector if k % 2 == 0 else nc.gpsimd
                    eng.tensor_scalar(kmT[:, k, :], kT_sb[:, k, :],
                                      mix_sb[:, k, hh:hh + 1], None,
                                      op0=mybir.AluOpType.mult)
                E_sb = headpool.tile([128, VT, SEQ], F16, tag="E")
                for t in range(VT):
                    ps = psA.tile([128, SEQ], F32, tag="psA")
                    for k in range(KT):
                        _mm(nc, ps[:], kmT[:, k, t * 128:(t + 1) * 128],
                            qT_sb[:, k, :], start=(k == 0), stop=(k == KT - 1))
                    nc.scalar.activation(E_sb[:, t, :], ps[:],
                                         mybir.ActivationFunctionType.Exp,
                                         bias=cbs_sb[:, t, hh:hh + 1],
                                         scale=1.0 / SCALE)
                psc = psB.tile([64, SEQ], F32, tag="psB")
                for t in range(VT):
                    _mm(nc, psc[:], vct_sb[:, t, hh * 64:(hh + 1) * 64],
                        E_sb[:, t, :], start=(t == 0), stop=(t == VT - 1))
                r_sb = headpool.tile([1, SEQ], F32, tag="r")
                nc.vector.reciprocal(r_sb[:], psc[32:33, :])
                r_h16 = headpool.tile([1, SEQ], F16, tag="r_h16")
                nc.vector.tensor_copy(r_h16[:], r_sb[:])
                rb_sb = headpool.tile([64, SEQ], F16, tag="rb")
                nc.gpsimd.partition_broadcast(rb_sb[:], r_h16[:])
                p0 = 64 * (hh % 2)
                nc.vector.tensor_mul(ctxT_sb[p0:p0 + 64, hh // 2, :],
                                     psc[:], rb_sb[:])
            hstack.close()

            # ---------------- epilogue: Wd, residual, LN, pooled mean -------
            estack = ExitStack()
            epool = estack.enter_context(tc.tile_pool(name="epi", bufs=2))
            o_ps = [psB.tile([1, 512], F32, tag="psB", name="o_ps0"),
                    psB.tile([1, 256], F32, tag="psB", name="o_ps1")]
            for ut in range(VT):
                x_sb = epool.tile([128, D], F32, tag="x")
                for ns, (c0, c1) in enumerate(((0, 512), (512, 768))):
                    ps = psA.tile([128, c1 - c0], F32, tag="psA")
                    for k in range(8):
                        _mm(nc, ps[:], ctxT_sb[:, k, ut * 128:(ut + 1) * 128],
                            wd_sb[:, k, c0:c1], start=(k == 0), stop=False)
                    _mm(nc, ps[:], ones_sb[:, ut * 128:(ut + 1) * 128],
                        bd_sb[:, c0:c1], start=False, stop=True)
                    nc.vector.tensor_add(x_sb[:, c0:c1], ps[:],
                                         h_nm_sb[:, ut, c0:c1])
                mu = epool.tile([128, 1], F32, tag="mu")
                nc.vector.reduce_sum(mu[:], x_sb[:], axis=mybir.AxisListType.X)
                nc.vector.tensor_scalar(mu[:], mu[:], 1.0 / D, None,
                                        op0=mybir.AluOpType.mult)
                xc_sb = epool.tile([128, D], F32, tag="xc")
                nc.vector.tensor_scalar(xc_sb[:], x_sb[:], mu[:], None,
                                        op0=mybir.AluOpType.subtract)
                sq_sb = epool.tile([128, D], F32, tag="sq")
                ssq = epool.tile([128, 1], F32, tag="ssq")
                nc.scalar.activation(sq_sb[:], xc_sb[:],
                                     mybir.ActivationFunctionType.Square,
                                     accum_out=ssq[:])
                nc.vector.tensor_scalar(ssq[:], ssq[:], 1.0 / D, EPS,
                                        op0=mybir.AluOpType.mult,
                                        op1=mybir.AluOpType.add)
                nc.scalar.sqrt(ssq[:], ssq[:])
                rstd = epool.tile([128, 1], F32, tag="rstd")
                nc.vector.reciprocal(rstd[:], ssq[:])
                t1_sb = epool.tile([128, D], F32, tag="t1")
                nc.gpsimd.tensor_scalar(t1_sb[:], xc_sb[:], rstd[:], None,
                                        op0=mybir.AluOpType.mult)
                t2_sb = epool.tile([128, D], F32, tag="t2")
                nc.gpsimd.tensor_mul(t2_sb[:], t1_sb[:], lng_b[:])
                y_h16 = epool.tile([128, D], F16, tag="y")
                nc.vector.tensor_add(y_h16[:], t2_sb[:], lnb_b[:])
                for ns, (c0, c1) in enumerate(((0, 512), (512, 768))):
                    _mm(nc, o_ps[ns][:], cf_h16[:, ut:ut + 1], y_h16[:, c0:c1],
                        start=(ut == 0), stop=(ut == VT - 1))
            estack.close()

            o_sb = cpool.tile([1, D], F32, tag="o_sb")
            for ns, (c0, c1) in enumerate(((0, 512), (512, 768))):
                nc.vector.tensor_scalar(o_sb[:, c0:c1], o_ps[ns][:], 1.0 / NE, None,
                                        op0=mybir.AluOpType.mult)
            nc.sync.dma_start(o_out[:], o_sb[:])

            dbg_sb = cpool.tile([128, 2 * VT], F32, tag="dbg_sb")
            nc.vector.tensor_copy(dbg_sb[:, 0:VT], ct_cols[:])
            nc.vector.tensor_copy(dbg_sb[:, VT:2 * VT], cf_cols[:])
            nc.sync.dma_start(dbg[:], dbg_sb[:])

    nc.finalize()   # Bacc: reg alloc, wait splitting, library loads, ISA codegen
    return nc


def _pad_wd(Wd, bd):
    """[PADK+1, 768] fp16 with rows permuted to the device ctx block layout
    [dims 0:32 | Z slot | dims 32:48 | 15 pad] per 64-row head block; the Z
    slot and pad rows are zero so the (scaled) Z row and padding contribute
    nothing.  Row PADK = bd."""
    WdT = np.asarray(Wd, np.float32).T
    out = np.zeros((PADK + 1, D), np.float16)
    for h in range(H):
        out[h * 64:h * 64 + 32, :] = WdT[h * DH:h * DH + 32, :]
        out[h * 64 + 33:h * 64 + 49, :] = WdT[h * DH + 32:(h + 1) * DH, :]
    out[PADK, :] = np.asarray(bd, np.float32)
    return out


def _core_inputs(h_b, fr, to, W):
    """Per-core in_map. W: dict with Wq,Wk,Wcb,Wv,bv,mix,Wd,bd,lng,lnb."""
    f16, f32 = np.float16, np.float32
    hT = np.asarray(h_b, f32).T
    return {
        "hTe": np.concatenate([hT, np.ones((1, SEQ), f32)], axis=0).astype(f16),
        "h_nm": np.ascontiguousarray(h_b, dtype=f32),
        "WqT": np.ascontiguousarray(np.asarray(W["Wq"], f32).T).astype(f16),
        "WkT": np.ascontiguousarray(np.asarray(W["Wk"], f32).T).astype(f16),
        "WvTe": np.concatenate(
            [np.asarray(W["Wv"], f32).T, np.asarray(W["bv"], f32)[None, :]],
            axis=0).astype(f16),
        "WdTe": _pad_wd(W["Wd"], W["bd"]),
        "WcbT": np.ascontiguousarray(np.asarray(W["Wcb"], f32).T).astype(f16),
        "mixT": np.ascontiguousarray(np.asarray(W["mix"], f32).T),
        "lng": np.asarray(W["lng"], f32)[None, :].astype(f16),
        "lnb": np.asarray(W["lnb"], f32)[None, :].astype(f16),
        "frpos": np.ascontiguousarray(fr, dtype=np.int32)[None, :],
        "topos": np.ascontiguousarray(to, dtype=np.int32)[None, :],
    }


def kernel(**inputs):
    hs = np.asarray(inputs["hidden_states"], dtype=np.float32)
    fpos = np.asarray(inputs["fpos"], dtype=np.int32)
    tpos = np.asarray(inputs["tpos"], dtype=np.int32)
    Wsets = {}
    for p in ("qtoc", "ctoq"):
        Wsets[p] = {n: np.asarray(inputs[p + "_" + n]) for n in
                    ("Wq", "Wk", "Wcb", "Wv", "Wd", "mix", "bv", "bd", "lng", "lnb")}

    # cores 0-5: the 6 unique (segment, direction) sub-problems;
    # cores 6-7: redundant duplicates so all 8 cores run the same program.
    tasks = [(b, d) for b in range(3) for d in ("qtoc", "ctoq")]
    tasks += [tasks[0], tasks[1]]
    in_maps = []
    for (b, d) in tasks:
        fr, to = (fpos[b], tpos[b]) if d == "qtoc" else (tpos[b], fpos[b])
        in_maps.append(_core_inputs(hs[b], fr, to, Wsets[d]))

    if "nc" not in _NC_CACHE:
        _NC_CACHE["nc"] = build_nc()
    nc = _NC_CACHE["nc"]
    res = run_bass_kernel_spmd(nc, in_maps, list(range(8)))
    results = res.results

    out = np.empty((3, 2 * D), np.float32)
    for c, (b, d) in enumerate(tasks[:6]):
        half = 0 if d == "qtoc" else 1
        out[b, half * D:(half + 1) * D] = results[c]["o"].reshape(D)
    return out


if __name__ == "__main__":
    import reference
    inp = reference.setup_inputs()
    got = kernel(**{k: np.asarray(v) for k, v in inp.items()})
    exp = np.asarray(reference.reference(**inp))
    print("rel err:", np.abs(got - exp).max() / np.abs(exp).max())



# revision 16
# speedup vs baseline: 4.0273x; 4.0273x over previous
"""Trainium2 Bass kernel for nn_Encoder_79096117723504 (gnn_message_passing).

Node-space collapse (see kernel_v0): every edge-level quantity is a gather of
a node-level one, so the [H,F,T] edge attention reduces to 512-node space with
histogram weights.  This version restructures the device program around
measured TRN2 engine costs:

  * fp8e4 DoubleRow matmuls (216 ns per 256-contraction x 512-free MM, 2x the
    fp16 rate) for the score path: q/k projections and all 16 per-head
    [512,512] score matmuls.  Host pre-scales Wq/Wk/Wcb by 64 so fp8 operand
    magnitudes sit in e4m3's normal range; the 1/4096 comes out in the exp
    scale.  The v/ctx/Wd path stays fp16 (fp8 there costs 6-8e-3 rel err).
  * histogram counts ct/cf computed on HOST (np.bincount); ln(ct) is folded
    into the exp bias (per-partition in the S^T layout), which also turns the
    softmax Z row into a plain ones-contraction and drops the ct multiplies.
  * ONE batched [16,512] reciprocal for all 16 heads' softmax normalizers
    (reciprocal is ~3.3 us regardless of partition count), with Z rows
    gathered by tiny SBUF DMAs and 1/Z re-broadcast via a K=2 PE matmul.
  * LayerNorm gain/bias applied on host after pooling (exact: pooling is
    linear), with rstd folded into the pooling weight vector.
  * gpsimd unused (measured 7.5 us per [128,512] op); elementwise work split
    DVE/ACT by measured rates (ts ~330 ns, ACT copy ~686 ns, exp ~573 ns).

Per core one (segment, direction) sub-problem; cores 6,7 duplicate 0,1.
"""
import math
import os
import sys

import numpy as np

for p in ('/opt/trn_rl_repo', '/root/.axon_site/_ro/trn_rl_repo'):
    if os.path.isdir(p) and p not in sys.path:
        sys.path.insert(0, p)

import concourse.bass as bass
import concourse.mybir as mybir
from concourse import bacc, tile
from concourse.bass_utils import run_bass_kernel_spmd

F32 = mybir.dt.float32
F16 = mybir.dt.float16
F8 = mybir.dt.float8e4
AF = mybir.ActivationFunctionType
ALU = mybir.AluOpType
DR = mybir.MatmulPerfMode.DoubleRow

D = 768
H = 16
DH = 48
SEQ = 512
NE = 2048
EPS = 1e-5
SCALE = math.sqrt(D / H)
WS = 64.0                  # host weight pre-scale for the fp8 score path
EXPSC = 1.0 / (WS * WS * SCALE)   # exp scale: S_psum = WS^2 * S_true
KT = 6
VT = 4
PADK = H * 64

_NC_CACHE = {}


def build_nc():
    nc = bacc.Bacc(None, target_bir_lowering=False)

    # -------- DRAM I/O (bound by position) ---------------------------------
    hT8d = nc.declare_dram_parameter("hT8", [D, SEQ], F8, isOutput=False)
    hT16d = nc.declare_dram_parameter("hT16", [D, SEQ], F16, isOutput=False)
    h_nmd = nc.declare_dram_parameter("h_nm", [SEQ, D], F32, isOutput=False)
    wq8d = nc.declare_dram_parameter("Wq8", [D, D], F8, isOutput=False)
    wk8d = nc.declare_dram_parameter("Wk8", [D, D], F8, isOutput=False)
    wcb8d = nc.declare_dram_parameter("Wcb8", [D, H], F8, isOutput=False)
    wv16d = nc.declare_dram_parameter("WvTe", [D + 1, D], F16, isOutput=False)
    wd16d = nc.declare_dram_parameter("WdTe", [PADK + 1, D], F16, isOutput=False)
    mixd = nc.declare_dram_parameter("mixT", [D, H], F32, isOutput=False)
    lnctd = nc.declare_dram_parameter("lnct", [128, VT], F32, isOutput=False)
    cf16d = nc.declare_dram_parameter("cf16", [128, VT], F16, isOutput=False)
    seld = nc.declare_dram_parameter("sel", [2, 128], F16, isOutput=False)
    o_out = nc.declare_dram_parameter("o", [1, D], F32, isOutput=True)

    with tile.TileContext(nc) as tc:
        with (
            tc.tile_pool(name="const", bufs=1) as cp,
            tc.tile_pool(name="psA", bufs=3, space="PSUM") as psA,
            tc.tile_pool(name="psB", bufs=2, space="PSUM") as psB,
            tc.tile_pool(name="psC", bufs=1, space="PSUM") as psC,
            tc.tile_pool(name="psD", bufs=1, space="PSUM") as psD,
        ):
            # ---------------- bulk loads ----------------
            hT8 = cp.tile([128, KT, SEQ], F8, tag="hT8")
            nc.sync.dma_start(hT8[:], hT8d[:, :].rearrange("(t p) u -> p t u", p=128))
            hT16 = cp.tile([128, KT, SEQ], F16, tag="hT16")
            nc.sync.dma_start(hT16[:], hT16d[:, :].rearrange("(t p) u -> p t u", p=128))
            h_nm = cp.tile([128, VT, D], F32, tag="h_nm")
            nc.sync.dma_start(h_nm[:], h_nmd[:, :].rearrange("(t p) d -> p t d", p=128))
            wq8 = cp.tile([128, KT, D], F8, tag="wq8")
            nc.sync.dma_start(wq8[:], wq8d[:, :].rearrange("(t p) d -> p t d", p=128))
            wk8 = cp.tile([128, KT, D], F8, tag="wk8")
            nc.sync.dma_start(wk8[:], wk8d[:, :].rearrange("(t p) d -> p t d", p=128))
            wcb8 = cp.tile([128, KT, H], F8, tag="wcb8")
            nc.sync.dma_start(wcb8[:], wcb8d[:, :].rearrange("(t p) h -> p t h", p=128))
            wv16 = cp.tile([128, KT, D], F16, tag="wv16")
            nc.sync.dma_start(wv16[:], wv16d[0:D, :].rearrange("(t p) d -> p t d", p=128))
            bv = cp.tile([1, D], F16, tag="bv")
            nc.sync.dma_start(bv[:], wv16d[D:D + 1, :])
            wd16 = cp.tile([128, 8, D], F16, tag="wd16")
            nc.sync.dma_start(wd16[:], wd16d[0:PADK, :].rearrange("(t p) d -> p t d", p=128))
            bd = cp.tile([1, D], F16, tag="bd")
            nc.sync.dma_start(bd[:], wd16d[PADK:PADK + 1, :])
            mix = cp.tile([128, KT, H], F32, tag="mix")
            nc.sync.dma_start(mix[:], mixd[:, :].rearrange("(t p) h -> p t h", p=128))
            lnct = cp.tile([128, VT], F32, tag="lnct")
            nc.sync.dma_start(lnct[:], lnctd[:])
            cf16 = cp.tile([128, VT], F16, tag="cf16")
            nc.sync.dma_start(cf16[:], cf16d[:])

            ones = cp.tile([1, SEQ], F16, tag="ones")
            nc.vector.memset(ones[:], 1.0)
            # sel: K=2 partition-pair broadcast weights for 1/Z replication
            sel = cp.tile([2, 128], F16, tag="sel")
            nc.sync.dma_start(sel[:], seld[:])

            # ---------------- q/k projections (fp8 DoubleRow) ----------------
            qT8 = cp.tile([128, KT, SEQ], F8, tag="qT8")
            kT16 = cp.tile([128, KT, SEQ], F16, tag="kT16")
            for m in range(KT):
                ps = psA.tile([128, SEQ], F32, tag="psA")
                for kp in range(3):
                    nc.tensor.matmul(ps[:], wq8[:, 2 * kp:2 * kp + 2, m * 128:(m + 1) * 128],
                                     hT8[:, 2 * kp:2 * kp + 2, :],
                                     start=(kp == 0), stop=(kp == 2), perf_mode=DR)
                nc.scalar.activation(qT8[:, m, :], ps[:], AF.Copy)
            for m in range(KT):
                ps = psA.tile([128, SEQ], F32, tag="psA")
                for kp in range(3):
                    nc.tensor.matmul(ps[:], wk8[:, 2 * kp:2 * kp + 2, m * 128:(m + 1) * 128],
                                     hT8[:, 2 * kp:2 * kp + 2, :],
                                     start=(kp == 0), stop=(kp == 2), perf_mode=DR)
                nc.vector.tensor_copy(kT16[:, m, :], ps[:])

            # ---------------- content bias + ln(ct) fold ----------------
            # cbs[v,t,h] = cb[v,h]/SCALE + ln(ct[v]); per-partition exp bias.
            cbs = cp.tile([128, VT, H], F32, tag="cbs")
            for t in range(VT):
                ps = psA.tile([128, H], F32, tag="psA", name=f"cb{t}")
                for k in range(KT):
                    nc.tensor.matmul(ps[:], hT8[:, k, t * 128:(t + 1) * 128],
                                     wcb8[:, k, :], start=(k == 0), stop=(k == KT - 1))
                nc.vector.tensor_scalar(cbs[:, t, :], ps[:], 1.0 / (WS * SCALE),
                                        lnct[:, t:t + 1],
                                        op0=ALU.mult, op1=ALU.add)

            # ---------------- v (fp16), padded head-block layout -------------
            # 64-block per head: [v 0:32 | Z-slot=1 | v 32:48 | 15 zeros]; the
            # ones in the Z slot make psc row 32 the softmax normalizer Z
            # (ct is folded into E via the ln(ct) exp bias).
            vct = cp.tile([128, VT, PADK], F16, tag="vct")
            nc.vector.memset(vct[:], 0.0)
            for t in range(VT):
                zslot = vct[:, t, :].rearrange("p (h c) -> p h c", c=64)[:, :, 32:33]
                nc.vector.memset(zslot, 1.0)
                for ns, (c0, c1) in enumerate(((0, 384), (384, 768))):
                    ps = psA.tile([128, 384], F32, tag="psA")
                    for k in range(KT):
                        nc.tensor.matmul(ps[:], hT16[:, k, t * 128:(t + 1) * 128],
                                         wv16[:, k, c0:c1], start=(k == 0), stop=False)
                    nc.tensor.matmul(ps[:], ones[:, t * 128:(t + 1) * 128],
                                     bv[:, c0:c1], start=False, stop=True)
                    blk = vct[:, t, 64 * 8 * ns:64 * 8 * (ns + 1)].rearrange(
                        "p (h c) -> p h c", c=64)
                    src = ps[:, :].rearrange("p (h c) -> p h c", c=DH)
                    nc.vector.tensor_copy(blk[:, :, 0:32], src[:, :, 0:32])
                    nc.vector.tensor_copy(blk[:, :, 33:49], src[:, :, 32:DH])

            # ---------------- attention heads ----------------
            ctxU = cp.tile([128, 8, SEQ], F16, tag="ctxU")   # unnormalized ctx
            ctxT = cp.tile([128, 8, SEQ], F16, tag="ctxT")   # normalized
            # Z rows: heads 0-7 at partitions 0-7, heads 8-15 at 32-39 (engine
            # ops need 32-aligned partition bases per batch)
            Zall = cp.tile([40, SEQ], F16, tag="Zall")
            rall = cp.tile([40, SEQ], F16, tag="rall")
            rpair = cp.tile([2, 8, SEQ], F16, tag="rpair")

            def zrow(hh):
                return hh if hh < 8 else 24 + hh

            lnZ = cp.tile([40, SEQ], F32, tag="lnZ")

            def normalize_half(nb):
                """Batch-normalize head pairs 4*nb..4*nb+3.  1/Z = exp(-ln Z)
                on ACT (same table set as the head-loop Exp, no reciprocal);
                re-broadcast 1/Z pairs via a K=2 matmul; f16 multiply on DVE."""
                r0 = 32 * nb
                nc.scalar.activation(lnZ[r0:r0 + 8, :], Zall[r0:r0 + 8, :], AF.Ln)
                nc.scalar.activation(rall[r0:r0 + 8, :], lnZ[r0:r0 + 8, :],
                                     AF.Exp, scale=-1.0)
                for hh in range(8 * nb, 8 * nb + 8):
                    nc.sync.dma_start(rpair[hh % 2:hh % 2 + 1, hh // 2, :],
                                      rall[zrow(hh):zrow(hh) + 1, :])
                for kk in range(4 * nb, 4 * nb + 4):
                    rb = psD.tile([128, SEQ], F32, tag="rb", name=f"rb{kk}")
                    nc.tensor.matmul(rb[:], sel[:, :], rpair[:, kk, :],
                                     start=True, stop=True)
                    nc.vector.tensor_mul(ctxT[:, kk, :], ctxU[:, kk, :], rb[:])

            with tc.tile_pool(name="head", bufs=2) as hp:
                for hpi in range(8):
                    psc = psB.tile([128, SEQ], F32, tag="psc")
                    Es = []
                    for sub in range(2):
                        hh = 2 * hpi + sub
                        kmT = hp.tile([128, KT, SEQ], F8, tag="kmT", bufs=3)
                        for k in range(KT):
                            nc.vector.tensor_scalar(kmT[:, k, :], kT16[:, k, :],
                                                    mix[:, k, hh:hh + 1], None,
                                                    op0=ALU.mult)
                        E16 = hp.tile([128, VT, SEQ], F16, tag="E", bufs=3)
                        for t in range(VT):
                            ps = psA.tile([128, SEQ], F32, tag="psA")
                            for kp in range(3):
                                nc.tensor.matmul(
                                    ps[:], kmT[:, 2 * kp:2 * kp + 2, t * 128:(t + 1) * 128],
                                    qT8[:, 2 * kp:2 * kp + 2, :],
                                    start=(kp == 0), stop=(kp == 2), perf_mode=DR)
                            nc.scalar.activation(E16[:, t, :], ps[:], AF.Exp,
                                                 bias=cbs[:, t, hh:hh + 1], scale=EXPSC)
                        Es.append(E16)
                    if hpi == 4:
                        normalize_half(0)   # overlaps pairs 5-7
                    # ctx matmuls after BOTH subs' scores: exp latency hidden
                    for sub in range(2):
                        hh = 2 * hpi + sub
                        p0 = 64 * sub
                        for t in range(VT):
                            nc.tensor.matmul(psc[p0:p0 + 64, :],
                                             vct[:, t, hh * 64:(hh + 1) * 64],
                                             Es[sub][:, t, :],
                                             start=(t == 0), stop=(t == VT - 1))
                    nc.vector.tensor_copy(ctxU[:, hpi, :], psc[:])
                    # gather the two Z rows (partitions 32 / 96) for batching
                    za, zb = zrow(2 * hpi), zrow(2 * hpi + 1)
                    nc.sync.dma_start(Zall[za:za + 1, :], ctxU[32:33, hpi, :])
                    nc.sync.dma_start(Zall[zb:zb + 1, :], ctxU[96:97, hpi, :])
                normalize_half(1)

            # ---------------- epilogue: Wd, residual, LN, pooled mean --------
            o_ps = [psC.tile([1, 512], F32, tag="ops0", name="ops0"),
                    psC.tile([1, 256], F32, tag="ops1", name="ops1")]
            with tc.tile_pool(name="epi", bufs=2) as ep:
                for ut in range(VT):
                    x = ep.tile([128, D], F32, tag="x")
                    for ns, (c0, c1) in enumerate(((0, 512), (512, 768))):
                        ps = psA.tile([128, c1 - c0], F32, tag="psA")
                        for k in range(8):
                            nc.tensor.matmul(ps[:], ctxT[:, k, ut * 128:(ut + 1) * 128],
                                             wd16[:, k, c0:c1], start=(k == 0), stop=False)
                        nc.tensor.matmul(ps[:], ones[:, ut * 128:(ut + 1) * 128],
                                         bd[:, c0:c1], start=False, stop=True)
                        nc.vector.tensor_add(x[:, c0:c1], ps[:], h_nm[:, ut, c0:c1])
                    negmu = ep.tile([128, 1], F32, tag="negmu")
                    nc.vector.reduce_sum(negmu[:], x[:], axis=mybir.AxisListType.X)
                    nc.vector.tensor_scalar(negmu[:], negmu[:], -1.0 / D, None,
                                            op0=ALU.mult)
                    sq = ep.tile([128, D], F16, tag="sq")
                    ssq = ep.tile([128, 1], F32, tag="ssq")
                    nc.scalar.activation(sq[:], x[:], AF.Square,
                                         bias=negmu[:], scale=1.0, accum_out=ssq[:])
                    nc.vector.tensor_scalar(ssq[:], ssq[:], 1.0 / D, EPS,
                                            op0=ALU.mult, op1=ALU.add)
                    nc.scalar.sqrt(ssq[:], ssq[:])
                    rstd = ep.tile([128, 1], F32, tag="rstd")
                    nc.vector.reciprocal(rstd[:], ssq[:])
                    xc16 = ep.tile([128, D], F16, tag="xc16")
                    nc.vector.tensor_scalar(xc16[:], x[:], negmu[:], None,
                                            op0=ALU.add)
                    w16 = ep.tile([128, 1], F16, tag="w16")
                    nc.vector.tensor_scalar(w16[:], cf16[:, ut:ut + 1], rstd[:],
                                            None, op0=ALU.mult)
                    for ns, (c0, c1) in enumerate(((0, 512), (512, 768))):
                        nc.tensor.matmul(o_ps[ns][:], w16[:], xc16[:, c0:c1],
                                         start=(ut == 0), stop=(ut == VT - 1))

            o_sb = cp.tile([1, D], F32, tag="o_sb")
            nc.vector.tensor_copy(o_sb[:, 0:512], o_ps[0][:])
            nc.vector.tensor_copy(o_sb[:, 512:768], o_ps[1][:])
            nc.sync.dma_start(o_out[:], o_sb[:])

    nc.finalize()
    return nc


def _pad_wd(Wd, bd):
    """[PADK+1, 768] fp16, rows permuted to the ctx block layout
    [dims 0:32 | Z slot | dims 32:48 | 15 pad] per 64-row head block; Z-slot
    and pad rows zero.  Row PADK = bd."""
    WdT = np.asarray(Wd, np.float32).T
    out = np.zeros((PADK + 1, D), np.float16)
    for h in range(H):
        out[h * 64:h * 64 + 32, :] = WdT[h * DH:h * DH + 32, :]
        out[h * 64 + 33:h * 64 + 49, :] = WdT[h * DH + 32:(h + 1) * DH, :]
    out[PADK, :] = np.asarray(bd, np.float32)
    return out


def _core_inputs(h_b, fr, to, W):
    f16, f32 = np.float16, np.float32
    f8 = mybir.dt.np(F8)
    h = np.asarray(h_b, f32)
    hT = np.ascontiguousarray(h.T)
    fr = np.asarray(fr, np.int64) % SEQ
    to = np.asarray(to, np.int64) % SEQ
    ct = np.bincount(to, minlength=SEQ).astype(f32)
    cf = np.bincount(fr, minlength=SEQ).astype(f32)
    lnct = np.where(ct > 0, np.log(np.maximum(ct, 1e-6)), -30.0).astype(f32)
    return {
        "hT8": hT.astype(f8),
        "hT16": hT.astype(f16),
        "h_nm": np.ascontiguousarray(h),
        "Wq8": np.ascontiguousarray(np.asarray(W["Wq"], f32).T * WS).astype(f8),
        "Wk8": np.ascontiguousarray(np.asarray(W["Wk"], f32).T * WS).astype(f8),
        "Wcb8": np.ascontiguousarray(np.asarray(W["Wcb"], f32).T * WS).astype(f8),
        "WvTe": np.concatenate(
            [np.asarray(W["Wv"], f32).T, np.asarray(W["bv"], f32)[None, :]],
            axis=0).astype(f16),
        "WdTe": _pad_wd(W["Wd"], W["bd"]),
        "mixT": np.ascontiguousarray(np.asarray(W["mix"], f32).T),
        "lnct": np.ascontiguousarray(lnct.reshape(VT, 128).T),
        "cf16": np.ascontiguousarray(cf.reshape(VT, 128).T).astype(f16),
        "sel": _SEL,
    }


_SEL = np.zeros((2, 128), np.float16)
_SEL[0, 0:64] = 1.0
_SEL[1, 64:128] = 1.0


def kernel(**inputs):
    hs = np.asarray(inputs["hidden_states"], dtype=np.float32)
    fpos = np.asarray(inputs["fpos"], dtype=np.int32)
    tpos = np.asarray(inputs["tpos"], dtype=np.int32)
    Wsets = {}
    for p in ("qtoc", "ctoq"):
        Wsets[p] = {n: np.asarray(inputs[p + "_" + n]) for n in
                    ("Wq", "Wk", "Wcb", "Wv", "Wd", "mix", "bv", "bd", "lng", "lnb")}

    tasks = [(b, d) for b in range(3) for d in ("qtoc", "ctoq")]
    tasks += [tasks[0], tasks[1]]
    in_maps = []
    for (b, d) in tasks:
        fr, to = (fpos[b], tpos[b]) if d == "qtoc" else (tpos[b], fpos[b])
        in_maps.append(_core_inputs(hs[b], fr, to, Wsets[d]))

    if "nc" not in _NC_CACHE:
        _NC_CACHE["nc"] = build_nc()
    nc = _NC_CACHE["nc"]
    res = run_bass_kernel_spmd(nc, in_maps, list(range(8)))
    results = res.results

    out = np.empty((3, 2 * D), np.float32)
    for c, (b, d) in enumerate(tasks[:6]):
        W = Wsets[d]
        p = results[c]["o"].reshape(D).astype(np.float32)
        y = np.asarray(W["lng"], np.float32) * (p / NE) + np.asarray(W["lnb"], np.float32)
        half = 0 if d == "qtoc" else 1
        out[b, half * D:(half + 1) * D] = y
    return out


if __name__ == "__main__":
    import reference
    inp = reference.setup_inputs()
    got = kernel(**{k: np.asarray(v) for k, v in inp.items()})
    exp = np.asarray(reference.reference(**inp))
    print("rel err:", np.abs(got - exp).max() / np.abs(exp).max())
